# revision 33
# baseline (speedup 1.0000x reference)
"""GCN layer (out = A @ x @ W, A sparse COO) on 8 Trainium2 NeuronCores.

Strategy (1D dest partitioning, x replicated):
  - Destinations (output rows) are sharded across the 8 cores; x is
    replicated to every core's HBM, the [64,64] weight is replicated.
  - Host-side preprocessing is pure indexing: edges are bucketed by
    (core, dest-block of 128 rows, source-chunk of 25000 rows), padded to a
    fixed per-segment slot capacity (max over cores, so one SPMD NEFF works
    for all 8 cores), and emitted as gather-index / value / dest-local
    streams.  Edges are sorted by source row within each segment for HBM
    locality.
  - Device per core: for each window of dest blocks, for each of the 4
    source chunks: dma_gather x rows (256B each) into SBUF -- gathers are
    issued round-robin over all 4 SWDGE queues so descriptor generation
    runs on all 8 GpSimd Q7 cores concurrently (4 pairs) instead of just
    pair 0; DVE multiplies the gathered rows by edge_val (bf16 out) and
    builds a one-hot [128 edges x 128 dest] bf16 matrix from dest-local
    ids (is_equal vs an iota row); PE matmuls (bf16, single-pass) accumulate
    aggT[64 feat, 128 dest] per block in fp32 PSUM across the window; at
    window end the [64,64] bf16 weight is applied per block
    (out_blk = (aggT)^T @ W) and fp32 results are DMA'd out.
  - Host concatenates the 8 output shards and truncates padding.
"""

import os
import numpy as np

# ---------------------------------------------------------------- config ---
class CFG:
    def __init__(self, n_nodes, d, n_cores, chunk, nchunks, nblk, window, sub):
        self.N = n_nodes
        self.D = d
        self.C = n_cores
        self.CHUNK = chunk          # x rows per gather chunk (< 32768 for int16)
        self.NCH = nchunks
        assert chunk * nchunks >= n_nodes
        self.NBLK = nblk            # dest blocks (of 128 rows) per core
        self.CORE_ROWS = 128 * nblk
        assert self.CORE_ROWS * n_cores >= n_nodes
        self.WINDOW = window        # blocks per window
        self.SUB = sub              # slots per DVE/PE subtile
        self.windows = [
            (w0, min(w0 + window, nblk)) for w0 in range(0, nblk, window)
        ]


FULL = CFG(n_nodes=100000, d=64, n_cores=8, chunk=25000, nchunks=4,
           nblk=98, window=6, sub=16)


# ---------------------------------------------------------- preprocessing ---
def preprocess(x, edge_row, edge_col, edge_val, cfg):
    """Bucket/pad edges; build per-core device input arrays.

    Returns (caps, plan, per_core_inputs):
      caps[b][k]   : slots (128-edge groups) for (block b, chunk k), shared
                     across cores.
      plan         : list over (w,k) of dicts with slot->block mapping etc.
      per_core     : list of dicts of numpy arrays keyed by dram tensor name.
    """
    C, NBLK, NCH = cfg.C, cfg.NBLK, cfg.NCH
    NPOS = C * NBLK
    r = edge_row.astype(np.int64)
    gblk = r // 128
    d = r % 128
    k = edge_col.astype(np.int64) // cfg.CHUNK
    lidx = (edge_col.astype(np.int64) % cfg.CHUNK).astype(np.int16)

    # Balance dest blocks across (core, position): slot capacities are
    # shared across cores (one SPMD NEFF, caps = max over the 8 blocks at
    # a position), so group blocks with similar per-chunk slot needs at
    # the same position (lexicographic sort of the cap vectors).
    gcounts = np.bincount(gblk * NCH + k, minlength=NPOS * NCH) \
        .reshape(NPOS, NCH)
    gcaps = -(-gcounts // 128)
    order_g = np.lexsort((gcaps[:, 3], gcaps[:, 2], gcaps[:, 1],
                          gcaps[:, 0]))
    A = order_g.reshape(NBLK, C)          # A[i, c] = global block id
    pos_c = np.empty(NPOS, np.int64)
    pos_i = np.empty(NPOS, np.int64)
    pos_c[A.reshape(-1)] = np.tile(np.arange(C), NBLK)
    pos_i[A.reshape(-1)] = np.repeat(np.arange(NBLK), C)
    caps = gcaps[A].max(axis=1)           # [NBLK, NCH]
    # every block must own >= 1 slot so its PSUM gets initialized
    empty = caps.sum(axis=1) == 0
    caps[empty, 0] = 1

    c = pos_c[gblk]
    b = pos_i[gblk]

    # order edges by (core, block, chunk, src) -- src-minor for DMA locality
    key = (((c * NBLK + b) * NCH + k) * cfg.CHUNK) + lidx
    order = np.argsort(key, kind="stable")
    seg_key = ((c * NBLK + b) * NCH + k)[order]
    counts = np.bincount(seg_key, minlength=C * NBLK * NCH).reshape(C, NBLK, NCH)
    assert (caps[None] >= np.ceil(counts / 128)).all()

    lidx_s = lidx[order]
    val_s = edge_val[order].astype(np.float32)
    d_s = d[order].astype(np.float32)

    # segment boundaries per (c,b,k) in the sorted stream
    starts = np.zeros(C * NBLK * NCH + 1, dtype=np.int64)
    np.cumsum(counts.reshape(-1), out=starts[1:])

    # fixed (w,k) segment sizes in slots
    plan = []
    for (w0, w1) in cfg.windows:
        for kk in range(NCH):
            nslots = int(caps[w0:w1, kk].sum())
            slot_block = np.repeat(np.arange(w0, w1), caps[w0:w1, kk])
            plan.append(dict(w0=w0, w1=w1, k=kk, nslots=nslots,
                             slot_block=slot_block))

    TOTS = sum(p["nslots"] for p in plan)

    per_core = []
    for cc in range(C):
        idx_mat = np.zeros((128, TOTS * 8), dtype=np.int16)
        val_mat = np.zeros((128, TOTS), dtype=np.float32)
        dst_mat = np.zeros((128, TOTS), dtype=np.float32)
        off = 0
        for p in plan:
            n = p["nslots"]
            if n == 0:
                continue
            kk = p["k"]
            # build padded segment arrays (length n*128)
            seg_idx = np.zeros(n * 128, dtype=np.int16)
            seg_val = np.zeros(n * 128, dtype=np.float32)
            seg_dst = np.zeros(n * 128, dtype=np.float32)
            pos = 0
            for bb in range(p["w0"], p["w1"]):
                gi = (cc * NBLK + bb) * NCH + kk
                s0, s1 = starts[gi], starts[gi + 1]
                cnt = s1 - s0
                blk_len = int(caps[bb, kk]) * 128
                seg_idx[pos:pos + cnt] = lidx_s[s0:s1]
                # trailing pad lanes get idx -1: dma_gather trims them at
                # runtime (per core), skipping their descriptors + DMA.
                # Keep >= 1 valid index: trimming to num_idxs == 0 wedges
                # the ucode.
                seg_idx[pos + cnt:pos + blk_len] = 0
                seg_val[pos:pos + cnt] = val_s[s0:s1]
                seg_dst[pos:pos + cnt] = d_s[s0:s1]
                pos += blk_len
            assert pos == n * 128
            # gather idx wrap: stream pos j -> (partition j%16, col j//16),
            # replicated into the 8 groups of 16 partitions
            iw = seg_idx.reshape(n * 8, 16).T          # [16, n*8]
            idx_mat[:, off * 8:(off + n) * 8] = np.tile(iw, (8, 1))
            # val/dest wrap: pos j -> (partition j%128, slot j//128)
            val_mat[:, off:off + n] = seg_val.reshape(n, 128).T
            dst_mat[:, off:off + n] = seg_dst.reshape(n, 128).T
            off += n
        per_core.append(dict(idx=idx_mat, val=val_mat, dst=dst_mat))

    return caps, plan, per_core, TOTS, A


# ---------------------------------------------------------------- kernel ---
def build_bass(cfg, caps, plan, TOTS):
    import concourse.bacc as bacc
    import concourse.bass as bass
    import concourse.mybir as mybir
    import concourse.tile as tile
    from concourse import library_config
    from concourse._compat import get_trn_type

    f32 = mybir.dt.float32
    bf16 = mybir.dt.bfloat16
    i16 = mybir.dt.int16
    D, NCH = cfg.D, cfg.NCH

    nc = bacc.Bacc(get_trn_type() or "TRN2", target_bir_lowering=False,
                   debug=False, num_swdge_queues=4)
    x_hbm = nc.dram_tensor("x", [cfg.CHUNK * NCH, D], f32,
                           kind="ExternalInput")
    w_hbm = nc.dram_tensor("w", [D, D], bf16, kind="ExternalInput")
    iota_hbm = nc.dram_tensor("iota", [128, 128], bf16, kind="ExternalInput")
    idx_hbm = nc.dram_tensor("idx", [128, TOTS * 8], i16,
                             kind="ExternalInput")
    val_hbm = nc.dram_tensor("val", [128, TOTS], f32, kind="ExternalInput")
    dst_hbm = nc.dram_tensor("dst", [128, TOTS], bf16, kind="ExternalInput")
    out_hbm = nc.dram_tensor("out", [cfg.CORE_ROWS, D], f32,
                             kind="ExternalOutput")

    # block -> (first (w,k,slot), last (w,k,slot)) for start/stop flags
    first_slot = {}
    last_slot = {}
    for pi, p in enumerate(plan):
        for s, bb in enumerate(p["slot_block"]):
            bb = int(bb)
            if bb not in first_slot:
                first_slot[bb] = (pi, s)
            last_slot[bb] = (pi, s)

    with tile.TileContext(nc) as tc:
        with (
            tc.tile_pool(name="const", bufs=1) as constp,
            tc.tile_pool(name="idxp", bufs=6) as idxp,
            tc.tile_pool(name="valp", bufs=6) as valp,
            tc.tile_pool(name="dstp", bufs=6) as dstp,
            tc.tile_pool(name="gp", bufs=6) as gp,
            tc.tile_pool(name="gvp", bufs=4) as gvp,
            tc.tile_pool(name="sp", bufs=4) as sp,
            tc.tile_pool(name="aggsb", bufs=4) as aggsbp,
            tc.tile_pool(name="stg", bufs=2) as stgp,
            tc.tile_pool(name="aggps", bufs=cfg.WINDOW,
                         space=bass.MemorySpace.PSUM) as aggpsp,
            tc.tile_pool(name="out2ps", bufs=2,
                         space=bass.MemorySpace.PSUM) as out2psp,
        ):
            nc.gpsimd.load_library(library_config.mlp)
            iota_sb = constp.tile([128, 128], bf16, tag="iota")
            w_sb = constp.tile([D, D], bf16, tag="w")
            nc.sync.dma_start(iota_sb[:], iota_hbm[:])
            nc.sync.dma_start(w_sb[:], w_hbm[:])

            nslots_max = max(p["nslots"] for p in plan)
            gq = 0  # SWDGE queue round-robin counter

            # touch every g buffer once so first-use lanes are never
            # uninitialized SBUF (NaN * 0 = NaN would poison the PSUM)
            for _ in range(6):
                gz = gp.tile([128, nslots_max, D], f32, tag="g")
                nc.vector.memset(gz[:], 0.0)

            for wi, (w0, w1) in enumerate(cfg.windows):
                nb = w1 - w0
                # one PSUM bank per block: accumulation-group state is
                # bank-wide, so two blocks must not share a bank
                aggps = [aggpsp.tile([64, 128], f32, tag="aggps",
                                     name=f"aggps_w{wi}_{i}")
                         for i in range(nb)]

                for kk in range(NCH):
                    p = plan[wi * NCH + kk]
                    n = p["nslots"]
                    if n == 0:
                        continue
                    off = sum(q["nslots"] for q in plan[:wi * NCH + kk])

                    idx_t = idxp.tile([128, nslots_max * 8], i16, tag="idx")
                    nc.sync.dma_start(idx_t[:, :n * 8],
                                      idx_hbm[:, off * 8:(off + n) * 8])
                    val_t = valp.tile([128, nslots_max], f32, tag="val")
                    nc.sync.dma_start(val_t[:, :n], val_hbm[:, off:off + n])
                    dst_t = dstp.tile([128, nslots_max], bf16, tag="dst")
                    nc.sync.dma_start(dst_t[:, :n], dst_hbm[:, off:off + n])
                    g_t = gp.tile([128, nslots_max, D], f32, tag="g")
                    # one gather call per dest block (finer calls
                    # interleave better across the 4 SWDGE queues than
                    # GMAX-packed ones; measured 649us vs 685us).
                    # Round-robin the queue so all four Q7 pairs generate
                    # descriptors in parallel.
                    q0 = 0
                    for bb in range(p["w0"], p["w1"]):
                        cb = int(caps[bb, kk])
                        while cb > 0:
                            step = min(cb, 4)
                            nq = step * 128
                            nc.gpsimd.dma_gather(
                                g_t[:, q0:q0 + step, :],
                                x_hbm[kk * cfg.CHUNK:(kk + 1) * cfg.CHUNK,
                                      :],
                                idx_t[:, q0 * 8:(q0 + step) * 8], nq, nq, D,
                                queue_num=gq % 4)
                            gq += 1
                            q0 += step
                            cb -= step
                    assert q0 == n

                    for s0 in range(0, n, cfg.SUB):
                        s1 = min(s0 + cfg.SUB, n)
                        ns = s1 - s0
                        gv_t = gvp.tile([128, cfg.SUB, D], bf16, tag="gv")
                        nc.vector.tensor_tensor(
                            gv_t[:, :ns, :], g_t[:, s0:s1, :],
                            val_t[:, s0:s1].unsqueeze(2)
                                .broadcast_to([128, ns, D]),
                            mybir.AluOpType.mult)
                        s_t = sp.tile([128, cfg.SUB, 128], bf16, tag="s")
                        nc.vector.tensor_tensor(
                            s_t[:, :ns, :],
                            dst_t[:, s0:s1].unsqueeze(2)
                                .broadcast_to([128, ns, 128]),
                            iota_sb[:, :].unsqueeze(1)
                                .broadcast_to([128, ns, 128]),
                            mybir.AluOpType.is_equal)
                        for s in range(s0, s1):
                            bb = int(p["slot_block"][s])
                            pi = wi * NCH + kk
                            st = first_slot[bb] == (pi, s)
                            sp_ = last_slot[bb] == (pi, s)
                            nc.tensor.matmul(
                                aggps[bb - w0][:, :],
                                gv_t[:, s - s0, :],
                                s_t[:, s - s0, :],
                                start=st, stop=sp_,
                                skip_group_check=True)

                # ---- flush window: apply W, stage, DMA out
                stg_t = stgp.tile([128, cfg.WINDOW, D], f32, tag="stg")
                out2 = out2psp.tile([128, cfg.WINDOW, D], f32, tag="out2")
                for bi in range(nb):
                    agg_sb = aggsbp.tile([64, 128], bf16, tag="aggsb",
                                         name=f"aggsb_w{wi}_{bi}")
                    nc.vector.tensor_copy(agg_sb[:, :], aggps[bi][:, :])
                    nc.tensor.matmul(out2[:, bi, :],
                                     agg_sb[:, :], w_sb[:],
                                     start=True, stop=True,
                                     skip_group_check=True)
                nc.vector.tensor_copy(stg_t[:, :nb, :], out2[:, :nb, :])
                # stg[p, b, f] -> out row (w0+b)*128+p, col f
                nc.sync.dma_start(
                    out_hbm[w0 * 128:w1 * 128, :]
                    .rearrange("(b p) f -> p b f", p=128),
                    stg_t[:, :nb, :])

    nc.compile()
    return nc


# ------------------------------------------------------------------- run ---
def _to_bf16(a):
    import ml_dtypes
    return a.astype(ml_dtypes.bfloat16)


def run(x, weight, edge_row, edge_col, edge_val, cfg=FULL, trace=False,
        trace_kwargs=None):
    from concourse.bass_utils import run_bass_kernel_spmd

    caps, plan, per_core, TOTS, A = preprocess(x, edge_row, edge_col,
                                               edge_val, cfg)
    nc = build_bass(cfg, caps, plan, TOTS)

    xpad = x
    if cfg.CHUNK * cfg.NCH > cfg.N:
        xpad = np.concatenate(
            [x, np.zeros((cfg.CHUNK * cfg.NCH - cfg.N, cfg.D),
                         dtype=np.float32)], axis=0)
    iota = _to_bf16(np.tile(np.arange(128, dtype=np.float32), (128, 1)))

    in_maps = []
    for cc in range(cfg.C):
        in_maps.append(dict(x=np.ascontiguousarray(xpad),
                            w=_to_bf16(weight),
                            iota=iota,
                            idx=per_core[cc]["idx"],
                            val=per_core[cc]["val"],
                            dst=_to_bf16(per_core[cc]["dst"])))
    kw = {}
    if trace:
        kw = dict(trace=True, trace_kwargs=trace_kwargs or {})
    res = run_bass_kernel_spmd(nc, in_maps, core_ids=list(range(cfg.C)), **kw)
    outs = [r["out"] for r in res.results]
    # un-permute: core c's position i holds global dest block A[i, c]
    full = np.zeros((cfg.C * cfg.NBLK * 128, cfg.D), dtype=np.float32)
    fullb = full.reshape(cfg.C * cfg.NBLK, 128, cfg.D)
    for cc in range(cfg.C):
        fullb[A[:, cc]] = outs[cc].reshape(cfg.NBLK, 128, cfg.D)
    return full[:cfg.N], res


def kernel(x, weight, edge_row, edge_col, edge_val):
    x = np.asarray(x, dtype=np.float32)
    weight = np.asarray(weight, dtype=np.float32)
    edge_row = np.asarray(edge_row, dtype=np.int32)
    edge_col = np.asarray(edge_col, dtype=np.int32)
    edge_val = np.asarray(edge_val, dtype=np.float32)
    out, _ = run(x, weight, edge_row, edge_col, edge_val, FULL)
    return out


# revision 34
# speedup vs baseline: 1.0253x; 1.0253x over previous
"""GCN layer (out = A @ x @ W, A sparse COO) on 8 Trainium2 NeuronCores.

Strategy (1D dest partitioning, x replicated):
  - Destinations (output rows) are sharded across the 8 cores; x is
    replicated to every core's HBM, the [64,64] weight is replicated.
  - Host-side preprocessing is pure indexing: edges are bucketed by
    (core, dest-block of 128 rows, source-chunk of 25000 rows), padded to a
    fixed per-segment slot capacity (max over cores, so one SPMD NEFF works
    for all 8 cores), and emitted as gather-index / value / dest-local
    streams.  Edges are sorted by source row within each segment for HBM
    locality.
  - Device per core: for each window of dest blocks, for each of the 4
    source chunks: dma_gather x rows (256B each) into SBUF -- gathers are
    issued round-robin over all 4 SWDGE queues so descriptor generation
    runs on all 8 GpSimd Q7 cores concurrently (4 pairs) instead of just
    pair 0; DVE multiplies the gathered rows by edge_val (bf16 out) and
    builds a one-hot [128 edges x 128 dest] bf16 matrix from dest-local
    ids (is_equal vs an iota row); PE matmuls (bf16, single-pass) accumulate
    aggT[64 feat, 128 dest] per block in fp32 PSUM across the window; at
    window end the [64,64] bf16 weight is applied per block
    (out_blk = (aggT)^T @ W) and fp32 results are DMA'd out.
  - Host concatenates the 8 output shards and truncates padding.
"""

import os
import numpy as np

# ---------------------------------------------------------------- config ---
class CFG:
    def __init__(self, n_nodes, d, n_cores, chunk, nchunks, nblk, window, sub):
        self.N = n_nodes
        self.D = d
        self.C = n_cores
        self.CHUNK = chunk          # x rows per gather chunk (< 32768 for int16)
        self.NCH = nchunks
        assert chunk * nchunks >= n_nodes
        self.NBLK = nblk            # dest blocks (of 128 rows) per core
        self.CORE_ROWS = 128 * nblk
        assert self.CORE_ROWS * n_cores >= n_nodes
        self.WINDOW = window        # blocks per window
        self.SUB = sub              # slots per DVE/PE subtile
        self.windows = [
            (w0, min(w0 + window, nblk)) for w0 in range(0, nblk, window)
        ]


FULL = CFG(n_nodes=100000, d=64, n_cores=8, chunk=25000, nchunks=4,
           nblk=98, window=6, sub=16)


# ---------------------------------------------------------- preprocessing ---
def preprocess(x, edge_row, edge_col, edge_val, cfg):
    """Bucket/pad edges; build per-core device input arrays.

    Returns (caps, plan, per_core_inputs):
      caps[b][k]   : slots (128-edge groups) for (block b, chunk k), shared
                     across cores.
      plan         : list over (w,k) of dicts with slot->block mapping etc.
      per_core     : list of dicts of numpy arrays keyed by dram tensor name.
    """
    C, NBLK, NCH = cfg.C, cfg.NBLK, cfg.NCH
    NPOS = C * NBLK
    r = edge_row.astype(np.int64)
    gblk = r // 128
    d = r % 128
    k = edge_col.astype(np.int64) // cfg.CHUNK
    lidx = (edge_col.astype(np.int64) % cfg.CHUNK).astype(np.int16)

    # Balance dest blocks across (core, position): slot capacities are
    # shared across cores (one SPMD NEFF, caps = max over the 8 blocks at
    # a position), so group blocks with similar per-chunk slot needs at
    # the same position (lexicographic sort of the cap vectors).
    gcounts = np.bincount(gblk * NCH + k, minlength=NPOS * NCH) \
        .reshape(NPOS, NCH)
    gcaps = -(-gcounts // 128)
    order_g = np.lexsort((gcaps[:, 3], gcaps[:, 2], gcaps[:, 1],
                          gcaps[:, 0]))
    A = order_g.reshape(NBLK, C)          # A[i, c] = global block id
    pos_c = np.empty(NPOS, np.int64)
    pos_i = np.empty(NPOS, np.int64)
    pos_c[A.reshape(-1)] = np.tile(np.arange(C), NBLK)
    pos_i[A.reshape(-1)] = np.repeat(np.arange(NBLK), C)
    caps = gcaps[A].max(axis=1)           # [NBLK, NCH]
    # every block must own >= 1 slot so its PSUM gets initialized
    empty = caps.sum(axis=1) == 0
    caps[empty, 0] = 1

    c = pos_c[gblk]
    b = pos_i[gblk]

    # order edges by (core, block, chunk, src) -- src-minor for DMA locality
    key = (((c * NBLK + b) * NCH + k) * cfg.CHUNK) + lidx
    order = np.argsort(key, kind="stable")
    seg_key = ((c * NBLK + b) * NCH + k)[order]
    counts = np.bincount(seg_key, minlength=C * NBLK * NCH).reshape(C, NBLK, NCH)
    assert (caps[None] >= np.ceil(counts / 128)).all()

    lidx_s = lidx[order]
    val_s = edge_val[order].astype(np.float32)
    d_s = d[order].astype(np.float32)

    # segment boundaries per (c,b,k) in the sorted stream
    starts = np.zeros(C * NBLK * NCH + 1, dtype=np.int64)
    np.cumsum(counts.reshape(-1), out=starts[1:])

    # fixed (w,k) segment sizes in slots
    plan = []
    for (w0, w1) in cfg.windows:
        for kk in range(NCH):
            nslots = int(caps[w0:w1, kk].sum())
            slot_block = np.repeat(np.arange(w0, w1), caps[w0:w1, kk])
            plan.append(dict(w0=w0, w1=w1, k=kk, nslots=nslots,
                             slot_block=slot_block))

    TOTS = sum(p["nslots"] for p in plan)

    per_core = []
    for cc in range(C):
        idx_mat = np.zeros((128, TOTS * 8), dtype=np.int16)
        val_mat = np.zeros((128, TOTS), dtype=np.float32)
        dst_mat = np.zeros((128, TOTS), dtype=np.float32)
        off = 0
        for p in plan:
            n = p["nslots"]
            if n == 0:
                continue
            kk = p["k"]
            # build padded segment arrays (length n*128)
            seg_idx = np.zeros(n * 128, dtype=np.int16)
            seg_val = np.zeros(n * 128, dtype=np.float32)
            seg_dst = np.zeros(n * 128, dtype=np.float32)
            pos = 0
            for bb in range(p["w0"], p["w1"]):
                gi = (cc * NBLK + bb) * NCH + kk
                s0, s1 = starts[gi], starts[gi + 1]
                cnt = s1 - s0
                blk_len = int(caps[bb, kk]) * 128
                seg_idx[pos:pos + cnt] = lidx_s[s0:s1]
                # trailing pad lanes get idx -1: dma_gather trims them at
                # runtime (per core), skipping their descriptors + DMA.
                # Keep >= 1 valid index: trimming to num_idxs == 0 wedges
                # the ucode.
                seg_idx[pos + cnt:pos + blk_len] = 0
                seg_val[pos:pos + cnt] = val_s[s0:s1]
                seg_dst[pos:pos + cnt] = d_s[s0:s1]
                pos += blk_len
            assert pos == n * 128
            # gather idx wrap: stream pos j -> (partition j%16, col j//16),
            # replicated into the 8 groups of 16 partitions
            iw = seg_idx.reshape(n * 8, 16).T          # [16, n*8]
            idx_mat[:, off * 8:(off + n) * 8] = np.tile(iw, (8, 1))
            # val/dest wrap: pos j -> (partition j%128, slot j//128)
            val_mat[:, off:off + n] = seg_val.reshape(n, 128).T
            dst_mat[:, off:off + n] = seg_dst.reshape(n, 128).T
            off += n
        per_core.append(dict(idx=idx_mat, val=val_mat, dst=dst_mat))

    return caps, plan, per_core, TOTS, A


# ---------------------------------------------------------------- kernel ---
def build_bass(cfg, caps, plan, TOTS):
    import concourse.bacc as bacc
    import concourse.bass as bass
    import concourse.mybir as mybir
    import concourse.tile as tile
    from concourse import library_config
    from concourse._compat import get_trn_type

    f32 = mybir.dt.float32
    bf16 = mybir.dt.bfloat16
    i16 = mybir.dt.int16
    D, NCH = cfg.D, cfg.NCH

    nc = bacc.Bacc(get_trn_type() or "TRN2", target_bir_lowering=False,
                   debug=False, num_swdge_queues=4)
    x_hbm = nc.dram_tensor("x", [cfg.CHUNK * NCH, D], f32,
                           kind="ExternalInput")
    w_hbm = nc.dram_tensor("w", [D, D], bf16, kind="ExternalInput")
    iota_hbm = nc.dram_tensor("iota", [128, 128], bf16, kind="ExternalInput")
    idx_hbm = nc.dram_tensor("idx", [128, TOTS * 8], i16,
                             kind="ExternalInput")
    val_hbm = nc.dram_tensor("val", [128, TOTS], f32, kind="ExternalInput")
    dst_hbm = nc.dram_tensor("dst", [128, TOTS], bf16, kind="ExternalInput")
    out_hbm = nc.dram_tensor("out", [cfg.CORE_ROWS, D], f32,
                             kind="ExternalOutput")

    # block -> (first (w,k,slot), last (w,k,slot)) for start/stop flags
    first_slot = {}
    last_slot = {}
    for pi, p in enumerate(plan):
        for s, bb in enumerate(p["slot_block"]):
            bb = int(bb)
            if bb not in first_slot:
                first_slot[bb] = (pi, s)
            last_slot[bb] = (pi, s)

    with tile.TileContext(nc) as tc:
        with (
            tc.tile_pool(name="const", bufs=1) as constp,
            tc.tile_pool(name="idxp", bufs=6) as idxp,
            tc.tile_pool(name="valp", bufs=6) as valp,
            tc.tile_pool(name="dstp", bufs=6) as dstp,
            tc.tile_pool(name="gp", bufs=6) as gp,
            tc.tile_pool(name="gvp", bufs=4) as gvp,
            tc.tile_pool(name="sp", bufs=4) as sp,
            tc.tile_pool(name="aggsb", bufs=4) as aggsbp,
            tc.tile_pool(name="stg", bufs=2) as stgp,
            tc.tile_pool(name="aggps", bufs=cfg.WINDOW,
                         space=bass.MemorySpace.PSUM) as aggpsp,
            tc.tile_pool(name="out2ps", bufs=2,
                         space=bass.MemorySpace.PSUM) as out2psp,
        ):
            nc.gpsimd.load_library(library_config.mlp)
            iota_sb = constp.tile([128, 128], bf16, tag="iota")
            w_sb = constp.tile([D, D], bf16, tag="w")
            nc.sync.dma_start(iota_sb[:], iota_hbm[:])
            nc.sync.dma_start(w_sb[:], w_hbm[:])

            nslots_max = max(p["nslots"] for p in plan)
            gq = 0  # SWDGE queue round-robin counter

            # touch every g buffer once so first-use lanes are never
            # uninitialized SBUF (NaN * 0 = NaN would poison the PSUM)
            for _ in range(6):
                gz = gp.tile([128, nslots_max, D], f32, tag="g")
                nc.vector.memset(gz[:], 0.0)

            for wi, (w0, w1) in enumerate(cfg.windows):
                nb = w1 - w0
                # one PSUM bank per block: accumulation-group state is
                # bank-wide, so two blocks must not share a bank
                aggps = [aggpsp.tile([64, 128], f32, tag="aggps",
                                     name=f"aggps_w{wi}_{i}")
                         for i in range(nb)]

                for kk in range(NCH):
                    p = plan[wi * NCH + kk]
                    n = p["nslots"]
                    if n == 0:
                        continue
                    off = sum(q["nslots"] for q in plan[:wi * NCH + kk])

                    idx_t = idxp.tile([128, nslots_max * 8], i16, tag="idx")
                    nc.sync.dma_start(idx_t[:, :n * 8],
                                      idx_hbm[:, off * 8:(off + n) * 8])
                    val_t = valp.tile([128, nslots_max], f32, tag="val")
                    nc.sync.dma_start(val_t[:, :n], val_hbm[:, off:off + n])
                    dst_t = dstp.tile([128, nslots_max], bf16, tag="dst")
                    nc.sync.dma_start(dst_t[:, :n], dst_hbm[:, off:off + n])
                    g_t = gp.tile([128, nslots_max, D], f32, tag="g")
                    # one gather call per dest block (finer calls
                    # interleave better across the 4 SWDGE queues than
                    # GMAX-packed ones; measured 649us vs 685us).
                    # Round-robin the queue so all four Q7 pairs generate
                    # descriptors in parallel.
                    q0 = 0
                    for bb in range(p["w0"], p["w1"]):
                        cb = int(caps[bb, kk])
                        if cb == 0:
                            continue
                        q1 = q0 + cb
                        nq = cb * 128
                        nc.gpsimd.dma_gather(
                            g_t[:, q0:q1, :],
                            x_hbm[kk * cfg.CHUNK:(kk + 1) * cfg.CHUNK, :],
                            idx_t[:, q0 * 8:q1 * 8], nq, nq, D,
                            queue_num=gq % 4, single_packet=False)
                        gq += 1
                        q0 = q1
                    assert q0 == n

                    for s0 in range(0, n, cfg.SUB):
                        s1 = min(s0 + cfg.SUB, n)
                        ns = s1 - s0
                        gv_t = gvp.tile([128, cfg.SUB, D], bf16, tag="gv")
                        nc.vector.tensor_tensor(
                            gv_t[:, :ns, :], g_t[:, s0:s1, :],
                            val_t[:, s0:s1].unsqueeze(2)
                                .broadcast_to([128, ns, D]),
                            mybir.AluOpType.mult)
                        s_t = sp.tile([128, cfg.SUB, 128], bf16, tag="s")
                        nc.vector.tensor_tensor(
                            s_t[:, :ns, :],
                            dst_t[:, s0:s1].unsqueeze(2)
                                .broadcast_to([128, ns, 128]),
                            iota_sb[:, :].unsqueeze(1)
                                .broadcast_to([128, ns, 128]),
                            mybir.AluOpType.is_equal)
                        for s in range(s0, s1):
                            bb = int(p["slot_block"][s])
                            pi = wi * NCH + kk
                            st = first_slot[bb] == (pi, s)
                            sp_ = last_slot[bb] == (pi, s)
                            nc.tensor.matmul(
                                aggps[bb - w0][:, :],
                                gv_t[:, s - s0, :],
                                s_t[:, s - s0, :],
                                start=st, stop=sp_,
                                skip_group_check=True)

                # ---- flush window: apply W, stage, DMA out
                stg_t = stgp.tile([128, cfg.WINDOW, D], f32, tag="stg")
                out2 = out2psp.tile([128, cfg.WINDOW, D], f32, tag="out2")
                for bi in range(nb):
                    agg_sb = aggsbp.tile([64, 128], bf16, tag="aggsb",
                                         name=f"aggsb_w{wi}_{bi}")
                    nc.vector.tensor_copy(agg_sb[:, :], aggps[bi][:, :])
                    nc.tensor.matmul(out2[:, bi, :],
                                     agg_sb[:, :], w_sb[:],
                                     start=True, stop=True,
                                     skip_group_check=True)
                nc.vector.tensor_copy(stg_t[:, :nb, :], out2[:, :nb, :])
                # stg[p, b, f] -> out row (w0+b)*128+p, col f
                nc.sync.dma_start(
                    out_hbm[w0 * 128:w1 * 128, :]
                    .rearrange("(b p) f -> p b f", p=128),
                    stg_t[:, :nb, :])

    nc.compile()
    return nc


# ------------------------------------------------------------------- run ---
def _to_bf16(a):
    import ml_dtypes
    return a.astype(ml_dtypes.bfloat16)


def run(x, weight, edge_row, edge_col, edge_val, cfg=FULL, trace=False,
        trace_kwargs=None):
    from concourse.bass_utils import run_bass_kernel_spmd

    caps, plan, per_core, TOTS, A = preprocess(x, edge_row, edge_col,
                                               edge_val, cfg)
    nc = build_bass(cfg, caps, plan, TOTS)

    xpad = x
    if cfg.CHUNK * cfg.NCH > cfg.N:
        xpad = np.concatenate(
            [x, np.zeros((cfg.CHUNK * cfg.NCH - cfg.N, cfg.D),
                         dtype=np.float32)], axis=0)
    iota = _to_bf16(np.tile(np.arange(128, dtype=np.float32), (128, 1)))

    in_maps = []
    for cc in range(cfg.C):
        in_maps.append(dict(x=np.ascontiguousarray(xpad),
                            w=_to_bf16(weight),
                            iota=iota,
                            idx=per_core[cc]["idx"],
                            val=per_core[cc]["val"],
                            dst=_to_bf16(per_core[cc]["dst"])))
    kw = {}
    if trace:
        kw = dict(trace=True, trace_kwargs=trace_kwargs or {})
    res = run_bass_kernel_spmd(nc, in_maps, core_ids=list(range(cfg.C)), **kw)
    outs = [r["out"] for r in res.results]
    # un-permute: core c's position i holds global dest block A[i, c]
    full = np.zeros((cfg.C * cfg.NBLK * 128, cfg.D), dtype=np.float32)
    fullb = full.reshape(cfg.C * cfg.NBLK, 128, cfg.D)
    for cc in range(cfg.C):
        fullb[A[:, cc]] = outs[cc].reshape(cfg.NBLK, 128, cfg.D)
    return full[:cfg.N], res


def kernel(x, weight, edge_row, edge_col, edge_val):
    x = np.asarray(x, dtype=np.float32)
    weight = np.asarray(weight, dtype=np.float32)
    edge_row = np.asarray(edge_row, dtype=np.int32)
    edge_col = np.asarray(edge_col, dtype=np.int32)
    edge_val = np.asarray(edge_val, dtype=np.float32)
    out, _ = run(x, weight, edge_row, edge_col, edge_val, FULL)
    return out


# revision 37
# speedup vs baseline: 1.2398x; 1.2092x over previous
"""GCN layer (out = A @ x @ W, A sparse COO) on 8 Trainium2 NeuronCores.

Strategy (1D dest partitioning, x replicated):
  - Destinations (output rows) are sharded across the 8 cores; x is
    replicated to every core's HBM, the [64,64] weight is replicated.
  - Host-side preprocessing is pure indexing: edges are bucketed by
    (core, dest-block of 128 rows, source-chunk of 25000 rows), padded to a
    fixed per-segment slot capacity (max over cores, so one SPMD NEFF works
    for all 8 cores), and emitted as gather-index / value / dest-local
    streams.  Edges are sorted by source row within each segment for HBM
    locality.
  - Device per core: for each window of dest blocks, for each of the 4
    source chunks: dma_gather x rows (256B each) into SBUF -- gathers are
    issued round-robin over all 4 SWDGE queues so descriptor generation
    runs on all 8 GpSimd Q7 cores concurrently (4 pairs) instead of just
    pair 0; DVE multiplies the gathered rows by edge_val (bf16 out) and
    builds a one-hot [128 edges x 128 dest] bf16 matrix from dest-local
    ids (is_equal vs an iota row); PE matmuls (bf16, single-pass) accumulate
    aggT[64 feat, 128 dest] per block in fp32 PSUM across the window; at
    window end the [64,64] bf16 weight is applied per block
    (out_blk = (aggT)^T @ W) and fp32 results are DMA'd out.
  - Host concatenates the 8 output shards and truncates padding.
"""

import os
import numpy as np

# ---------------------------------------------------------------- config ---
class CFG:
    def __init__(self, n_nodes, d, n_cores, chunk, nchunks, nblk, window, sub):
        self.N = n_nodes
        self.D = d
        self.C = n_cores
        self.CHUNK = chunk          # x rows per gather chunk (< 32768 for int16)
        self.NCH = nchunks
        assert chunk * nchunks >= n_nodes
        self.NBLK = nblk            # dest blocks (of 128 rows) per core
        self.CORE_ROWS = 128 * nblk
        assert self.CORE_ROWS * n_cores >= n_nodes
        self.WINDOW = window        # blocks per window
        self.SUB = sub              # slots per DVE/PE subtile
        self.windows = [
            (w0, min(w0 + window, nblk)) for w0 in range(0, nblk, window)
        ]


FULL = CFG(n_nodes=100000, d=64, n_cores=8, chunk=25000, nchunks=4,
           nblk=98, window=6, sub=16)


# ---------------------------------------------------------- preprocessing ---
def preprocess(x, edge_row, edge_col, edge_val, cfg):
    """Bucket/pad edges; build per-core device input arrays.

    Returns (caps, plan, per_core_inputs):
      caps[b][k]   : slots (128-edge groups) for (block b, chunk k), shared
                     across cores.
      plan         : list over (w,k) of dicts with slot->block mapping etc.
      per_core     : list of dicts of numpy arrays keyed by dram tensor name.
    """
    C, NBLK, NCH = cfg.C, cfg.NBLK, cfg.NCH
    NPOS = C * NBLK
    r = edge_row.astype(np.int64)
    gblk = r // 128
    d = r % 128
    k = edge_col.astype(np.int64) // cfg.CHUNK
    lidx = (edge_col.astype(np.int64) % cfg.CHUNK).astype(np.int16)

    # Balance dest blocks across (core, position): slot capacities are
    # shared across cores (one SPMD NEFF, caps = max over the 8 blocks at
    # a position), so group blocks with similar per-chunk slot needs at
    # the same position (lexicographic sort of the cap vectors).
    gcounts = np.bincount(gblk * NCH + k, minlength=NPOS * NCH) \
        .reshape(NPOS, NCH)
    gcaps = -(-gcounts // 128)
    order_g = np.lexsort((gcaps[:, 3], gcaps[:, 2], gcaps[:, 1],
                          gcaps[:, 0]))
    A = order_g.reshape(NBLK, C)          # A[i, c] = global block id
    pos_c = np.empty(NPOS, np.int64)
    pos_i = np.empty(NPOS, np.int64)
    pos_c[A.reshape(-1)] = np.tile(np.arange(C), NBLK)
    pos_i[A.reshape(-1)] = np.repeat(np.arange(NBLK), C)
    caps = gcaps[A].max(axis=1)           # [NBLK, NCH]
    # every block must own >= 1 slot so its PSUM gets initialized
    empty = caps.sum(axis=1) == 0
    caps[empty, 0] = 1

    c = pos_c[gblk]
    b = pos_i[gblk]

    # order edges by (core, block, chunk, src) -- src-minor for DMA locality
    key = (((c * NBLK + b) * NCH + k) * cfg.CHUNK) + lidx
    order = np.argsort(key, kind="stable")
    seg_key = ((c * NBLK + b) * NCH + k)[order]
    counts = np.bincount(seg_key, minlength=C * NBLK * NCH).reshape(C, NBLK, NCH)
    assert (caps[None] >= np.ceil(counts / 128)).all()

    lidx_s = lidx[order]
    val_s = edge_val[order].astype(np.float32)
    d_s = d[order].astype(np.float32)

    # segment boundaries per (c,b,k) in the sorted stream
    starts = np.zeros(C * NBLK * NCH + 1, dtype=np.int64)
    np.cumsum(counts.reshape(-1), out=starts[1:])

    # fixed (w,k) segment sizes in slots
    plan = []
    for (w0, w1) in cfg.windows:
        for kk in range(NCH):
            nslots = int(caps[w0:w1, kk].sum())
            slot_block = np.repeat(np.arange(w0, w1), caps[w0:w1, kk])
            plan.append(dict(w0=w0, w1=w1, k=kk, nslots=nslots,
                             slot_block=slot_block))

    TOTS = sum(p["nslots"] for p in plan)

    per_core = []
    for cc in range(C):
        idx_mat = np.zeros((128, TOTS * 8), dtype=np.int16)
        val_mat = np.zeros((128, TOTS), dtype=np.float32)
        dst_mat = np.zeros((128, TOTS), dtype=np.float32)
        off = 0
        for p in plan:
            n = p["nslots"]
            if n == 0:
                continue
            kk = p["k"]
            # build padded segment arrays (length n*128)
            seg_idx = np.zeros(n * 128, dtype=np.int16)
            seg_val = np.zeros(n * 128, dtype=np.float32)
            seg_dst = np.zeros(n * 128, dtype=np.float32)
            pos = 0
            for bb in range(p["w0"], p["w1"]):
                gi = (cc * NBLK + bb) * NCH + kk
                s0, s1 = starts[gi], starts[gi + 1]
                cnt = s1 - s0
                blk_len = int(caps[bb, kk]) * 128
                seg_idx[pos:pos + cnt] = lidx_s[s0:s1]
                # trailing pad lanes get idx -1: dma_gather trims them at
                # runtime (per core), skipping their descriptors + DMA.
                # Keep >= 1 valid index: trimming to num_idxs == 0 wedges
                # the ucode.
                seg_idx[pos + cnt:pos + blk_len] = 0
                seg_val[pos:pos + cnt] = val_s[s0:s1]
                seg_dst[pos:pos + cnt] = d_s[s0:s1]
                pos += blk_len
            assert pos == n * 128
            # gather idx wrap: stream pos j -> (partition j%16, col j//16),
            # replicated into the 8 groups of 16 partitions
            iw = seg_idx.reshape(n * 8, 16).T          # [16, n*8]
            idx_mat[:, off * 8:(off + n) * 8] = np.tile(iw, (8, 1))
            # val/dest wrap: pos j -> (partition j%128, slot j//128)
            val_mat[:, off:off + n] = seg_val.reshape(n, 128).T
            dst_mat[:, off:off + n] = seg_dst.reshape(n, 128).T
            off += n
        per_core.append(dict(idx=idx_mat, val=val_mat, dst=dst_mat))

    return caps, plan, per_core, TOTS, A


# ---------------------------------------------------------------- kernel ---
def build_bass(cfg, caps, plan, TOTS):
    import concourse.bacc as bacc
    import concourse.bass as bass
    import concourse.mybir as mybir
    import concourse.tile as tile
    from concourse import library_config
    from concourse._compat import get_trn_type

    f32 = mybir.dt.float32
    bf16 = mybir.dt.bfloat16
    i16 = mybir.dt.int16
    D, NCH = cfg.D, cfg.NCH

    nc = bacc.Bacc(get_trn_type() or "TRN2", target_bir_lowering=False,
                   debug=False, num_swdge_queues=4)
    x_hbm = nc.dram_tensor("x", [cfg.CHUNK * NCH, D], f32,
                           kind="ExternalInput")
    w_hbm = nc.dram_tensor("w", [D, D], bf16, kind="ExternalInput")
    iota_hbm = nc.dram_tensor("iota", [128, 128], bf16, kind="ExternalInput")
    idx_hbm = nc.dram_tensor("idx", [128, TOTS * 8], i16,
                             kind="ExternalInput")
    val_hbm = nc.dram_tensor("val", [128, TOTS], f32, kind="ExternalInput")
    dst_hbm = nc.dram_tensor("dst", [128, TOTS], bf16, kind="ExternalInput")
    out_hbm = nc.dram_tensor("out", [cfg.CORE_ROWS, D], f32,
                             kind="ExternalOutput")

    # block -> (first (w,k,slot), last (w,k,slot)) for start/stop flags
    first_slot = {}
    last_slot = {}
    for pi, p in enumerate(plan):
        for s, bb in enumerate(p["slot_block"]):
            bb = int(bb)
            if bb not in first_slot:
                first_slot[bb] = (pi, s)
            last_slot[bb] = (pi, s)

    with tile.TileContext(nc) as tc:
        with (
            tc.tile_pool(name="const", bufs=1) as constp,
            tc.tile_pool(name="idxp", bufs=5) as idxp,
            tc.tile_pool(name="valp", bufs=5) as valp,
            tc.tile_pool(name="dstp", bufs=5) as dstp,
            tc.tile_pool(name="gp", bufs=5) as gp,
            tc.tile_pool(name="gvp", bufs=4) as gvp,
            tc.tile_pool(name="sp", bufs=4) as sp,
            tc.tile_pool(name="aggsb", bufs=4) as aggsbp,
            tc.tile_pool(name="stg", bufs=2) as stgp,
            tc.tile_pool(name="aggps", bufs=cfg.WINDOW,
                         space=bass.MemorySpace.PSUM) as aggpsp,
            tc.tile_pool(name="out2ps", bufs=2,
                         space=bass.MemorySpace.PSUM) as out2psp,
        ):
            nc.gpsimd.load_library(library_config.mlp)
            iota_sb = constp.tile([128, 128], bf16, tag="iota")
            w_sb = constp.tile([D, D], bf16, tag="w")
            nc.sync.dma_start(iota_sb[:], iota_hbm[:])
            nc.sync.dma_start(w_sb[:], w_hbm[:])

            nslots_max = max(p["nslots"] for p in plan)
            gq = 0  # SWDGE queue round-robin counter

            # touch every g buffer once so first-use lanes are never
            # uninitialized SBUF (NaN * 0 = NaN would poison the PSUM)
            for _ in range(5):
                gz = gp.tile([128, nslots_max, D], f32, tag="g")
                nc.vector.memset(gz[:], 0.0)

            for wi, (w0, w1) in enumerate(cfg.windows):
                nb = w1 - w0
                # one PSUM bank per block: accumulation-group state is
                # bank-wide, so two blocks must not share a bank
                aggps = [aggpsp.tile([64, 128], f32, tag="aggps",
                                     name=f"aggps_w{wi}_{i}")
                         for i in range(nb)]

                for kk in range(NCH):
                    p = plan[wi * NCH + kk]
                    n = p["nslots"]
                    if n == 0:
                        continue
                    off = sum(q["nslots"] for q in plan[:wi * NCH + kk])

                    idx_t = idxp.tile([128, nslots_max * 8], i16, tag="idx")
                    nc.sync.dma_start(idx_t[:, :n * 8],
                                      idx_hbm[:, off * 8:(off + n) * 8])
                    val_t = valp.tile([128, nslots_max], f32, tag="val")
                    nc.sync.dma_start(val_t[:, :n], val_hbm[:, off:off + n])
                    dst_t = dstp.tile([128, nslots_max], bf16, tag="dst")
                    nc.sync.dma_start(dst_t[:, :n], dst_hbm[:, off:off + n])
                    g_t = gp.tile([128, nslots_max, D], f32, tag="g")
                    # one gather call per dest block (finer calls
                    # interleave better across the 4 SWDGE queues than
                    # GMAX-packed ones; measured 649us vs 685us).
                    # Round-robin the queue so all four Q7 pairs generate
                    # descriptors in parallel.
                    q0 = 0
                    for bb in range(p["w0"], p["w1"]):
                        cb = int(caps[bb, kk])
                        if cb == 0:
                            continue
                        q1 = q0 + cb
                        nq = cb * 128
                        nc.gpsimd.dma_gather(
                            g_t[:, q0:q1, :],
                            x_hbm[kk * cfg.CHUNK:(kk + 1) * cfg.CHUNK, :],
                            idx_t[:, q0 * 8:q1 * 8], nq, nq, D,
                            queue_num=gq % 4)
                        gq += 1
                        q0 = q1
                    assert q0 == n

                    for s0 in range(0, n, cfg.SUB):
                        s1 = min(s0 + cfg.SUB, n)
                        ns = s1 - s0
                        gv_t = gvp.tile([128, cfg.SUB, D], bf16, tag="gv")
                        nc.vector.tensor_tensor(
                            gv_t[:, :ns, :], g_t[:, s0:s1, :],
                            val_t[:, s0:s1].unsqueeze(2)
                                .broadcast_to([128, ns, D]),
                            mybir.AluOpType.mult)
                        s_t = sp.tile([128, cfg.SUB, 128], bf16, tag="s")
                        nc.vector.tensor_tensor(
                            s_t[:, :ns, :],
                            dst_t[:, s0:s1].unsqueeze(2)
                                .broadcast_to([128, ns, 128]),
                            iota_sb[:, :].unsqueeze(1)
                                .broadcast_to([128, ns, 128]),
                            mybir.AluOpType.is_equal)
                        for s in range(s0, s1):
                            bb = int(p["slot_block"][s])
                            pi = wi * NCH + kk
                            st = first_slot[bb] == (pi, s)
                            sp_ = last_slot[bb] == (pi, s)
                            nc.tensor.matmul(
                                aggps[bb - w0][:, :],
                                gv_t[:, s - s0, :],
                                s_t[:, s - s0, :],
                                start=st, stop=sp_,
                                skip_group_check=True)

                # ---- flush window: apply W, stage, DMA out
                stg_t = stgp.tile([128, cfg.WINDOW, D], f32, tag="stg")
                out2 = out2psp.tile([128, cfg.WINDOW, D], f32, tag="out2")
                for bi in range(nb):
                    agg_sb = aggsbp.tile([64, 128], bf16, tag="aggsb",
                                         name=f"aggsb_w{wi}_{bi}")
                    nc.vector.tensor_copy(agg_sb[:, :], aggps[bi][:, :])
                    nc.tensor.matmul(out2[:, bi, :],
                                     agg_sb[:, :], w_sb[:],
                                     start=True, stop=True,
                                     skip_group_check=True)
                nc.vector.tensor_copy(stg_t[:, :nb, :], out2[:, :nb, :])
                # stg[p, b, f] -> out row (w0+b)*128+p, col f
                nc.sync.dma_start(
                    out_hbm[w0 * 128:w1 * 128, :]
                    .rearrange("(b p) f -> p b f", p=128),
                    stg_t[:, :nb, :])

    nc.compile()
    return nc


# ------------------------------------------------------------------- run ---
def _to_bf16(a):
    import ml_dtypes
    return a.astype(ml_dtypes.bfloat16)


def run(x, weight, edge_row, edge_col, edge_val, cfg=FULL, trace=False,
        trace_kwargs=None):
    from concourse.bass_utils import run_bass_kernel_spmd

    caps, plan, per_core, TOTS, A = preprocess(x, edge_row, edge_col,
                                               edge_val, cfg)
    nc = build_bass(cfg, caps, plan, TOTS)

    xpad = x
    if cfg.CHUNK * cfg.NCH > cfg.N:
        xpad = np.concatenate(
            [x, np.zeros((cfg.CHUNK * cfg.NCH - cfg.N, cfg.D),
                         dtype=np.float32)], axis=0)
    iota = _to_bf16(np.tile(np.arange(128, dtype=np.float32), (128, 1)))

    in_maps = []
    for cc in range(cfg.C):
        in_maps.append(dict(x=np.ascontiguousarray(xpad),
                            w=_to_bf16(weight),
                            iota=iota,
                            idx=per_core[cc]["idx"],
                            val=per_core[cc]["val"],
                            dst=_to_bf16(per_core[cc]["dst"])))
    kw = {}
    if trace:
        kw = dict(trace=True, trace_kwargs=trace_kwargs or {})
    res = run_bass_kernel_spmd(nc, in_maps, core_ids=list(range(cfg.C)), **kw)
    outs = [r["out"] for r in res.results]
    # un-permute: core c's position i holds global dest block A[i, c]
    full = np.zeros((cfg.C * cfg.NBLK * 128, cfg.D), dtype=np.float32)
    fullb = full.reshape(cfg.C * cfg.NBLK, 128, cfg.D)
    for cc in range(cfg.C):
        fullb[A[:, cc]] = outs[cc].reshape(cfg.NBLK, 128, cfg.D)
    return full[:cfg.N], res


def kernel(x, weight, edge_row, edge_col, edge_val):
    x = np.asarray(x, dtype=np.float32)
    weight = np.asarray(weight, dtype=np.float32)
    edge_row = np.asarray(edge_row, dtype=np.int32)
    edge_col = np.asarray(edge_col, dtype=np.int32)
    edge_val = np.asarray(edge_val, dtype=np.float32)
    out, _ = run(x, weight, edge_row, edge_col, edge_val, FULL)
    return out


# revision 39
# speedup vs baseline: 1.3018x; 1.0500x over previous
"""GCN layer (out = A @ x @ W, A sparse COO) on 8 Trainium2 NeuronCores.

Strategy (1D dest partitioning, x replicated):
  - Destinations (output rows) are sharded across the 8 cores; x is
    replicated to every core's HBM, the [64,64] weight is replicated.
  - Host-side preprocessing is pure indexing: edges are bucketed by
    (core, dest-block of 128 rows, source-chunk of 25000 rows), padded to a
    fixed per-segment slot capacity (max over cores, so one SPMD NEFF works
    for all 8 cores), and emitted as gather-index / value / dest-local
    streams.  Edges are sorted by source row within each segment for HBM
    locality.
  - Device per core: for each window of dest blocks, for each of the 4
    source chunks: dma_gather x rows (256B each) into SBUF -- gathers are
    issued round-robin over all 4 SWDGE queues so descriptor generation
    runs on all 8 GpSimd Q7 cores concurrently (4 pairs) instead of just
    pair 0; DVE multiplies the gathered rows by edge_val (bf16 out) and
    builds a one-hot [128 edges x 128 dest] bf16 matrix from dest-local
    ids (is_equal vs an iota row); PE matmuls (bf16, single-pass) accumulate
    aggT[64 feat, 128 dest] per block in fp32 PSUM across the window; at
    window end the [64,64] bf16 weight is applied per block
    (out_blk = (aggT)^T @ W) and fp32 results are DMA'd out.
  - Host concatenates the 8 output shards and truncates padding.
"""

import os
import numpy as np

# ---------------------------------------------------------------- config ---
class CFG:
    def __init__(self, n_nodes, d, n_cores, chunk, nchunks, nblk, window, sub):
        self.N = n_nodes
        self.D = d
        self.C = n_cores
        self.CHUNK = chunk          # x rows per gather chunk (< 32768 for int16)
        self.NCH = nchunks
        assert chunk * nchunks >= n_nodes
        self.NBLK = nblk            # dest blocks (of 128 rows) per core
        self.CORE_ROWS = 128 * nblk
        assert self.CORE_ROWS * n_cores >= n_nodes
        self.WINDOW = window        # blocks per window
        self.SUB = sub              # slots per DVE/PE subtile
        self.windows = [
            (w0, min(w0 + window, nblk)) for w0 in range(0, nblk, window)
        ]


FULL = CFG(n_nodes=100000, d=64, n_cores=8, chunk=25000, nchunks=4,
           nblk=98, window=6, sub=16)


# ---------------------------------------------------------- preprocessing ---
def preprocess(x, edge_row, edge_col, edge_val, cfg):
    """Bucket/pad edges; build per-core device input arrays.

    Returns (caps, plan, per_core_inputs):
      caps[b][k]   : slots (128-edge groups) for (block b, chunk k), shared
                     across cores.
      plan         : list over (w,k) of dicts with slot->block mapping etc.
      per_core     : list of dicts of numpy arrays keyed by dram tensor name.
    """
    C, NBLK, NCH = cfg.C, cfg.NBLK, cfg.NCH
    NPOS = C * NBLK
    r = edge_row.astype(np.int64)
    gblk = r // 128
    d = r % 128
    k = edge_col.astype(np.int64) // cfg.CHUNK
    lidx = (edge_col.astype(np.int64) % cfg.CHUNK).astype(np.int16)

    # Balance dest blocks across (core, position): slot capacities are
    # shared across cores (one SPMD NEFF, caps = max over the 8 blocks at
    # a position), so group blocks with similar per-chunk slot needs at
    # the same position (lexicographic sort of the cap vectors).
    gcounts = np.bincount(gblk * NCH + k, minlength=NPOS * NCH) \
        .reshape(NPOS, NCH)
    gcaps = -(-gcounts // 128)
    order_g = np.lexsort((gcaps[:, 3], gcaps[:, 2], gcaps[:, 1],
                          gcaps[:, 0]))
    A = order_g.reshape(NBLK, C)          # A[i, c] = global block id
    pos_c = np.empty(NPOS, np.int64)
    pos_i = np.empty(NPOS, np.int64)
    pos_c[A.reshape(-1)] = np.tile(np.arange(C), NBLK)
    pos_i[A.reshape(-1)] = np.repeat(np.arange(NBLK), C)
    caps = gcaps[A].max(axis=1)           # [NBLK, NCH]
    # every block must own >= 1 slot so its PSUM gets initialized
    empty = caps.sum(axis=1) == 0
    caps[empty, 0] = 1

    c = pos_c[gblk]
    b = pos_i[gblk]

    # order edges by (core, block, chunk, src) -- src-minor for DMA locality
    key = (((c * NBLK + b) * NCH + k) * cfg.CHUNK) + lidx
    order = np.argsort(key, kind="stable")
    seg_key = ((c * NBLK + b) * NCH + k)[order]
    counts = np.bincount(seg_key, minlength=C * NBLK * NCH).reshape(C, NBLK, NCH)
    assert (caps[None] >= np.ceil(counts / 128)).all()

    lidx_s = lidx[order]
    val_s = edge_val[order].astype(np.float32)
    d_s = d[order].astype(np.float32)

    # segment boundaries per (c,b,k) in the sorted stream
    starts = np.zeros(C * NBLK * NCH + 1, dtype=np.int64)
    np.cumsum(counts.reshape(-1), out=starts[1:])

    # fixed (w,k) segment sizes in slots
    plan = []
    for (w0, w1) in cfg.windows:
        for kk in range(NCH):
            nslots = int(caps[w0:w1, kk].sum())
            slot_block = np.repeat(np.arange(w0, w1), caps[w0:w1, kk])
            plan.append(dict(w0=w0, w1=w1, k=kk, nslots=nslots,
                             slot_block=slot_block))

    TOTS = sum(p["nslots"] for p in plan)

    per_core = []
    for cc in range(C):
        idx_mat = np.zeros((128, TOTS * 8), dtype=np.int16)
        val_mat = np.zeros((128, TOTS), dtype=np.float32)
        dst_mat = np.zeros((128, TOTS), dtype=np.float32)
        off = 0
        for p in plan:
            n = p["nslots"]
            if n == 0:
                continue
            kk = p["k"]
            # build padded segment arrays (length n*128)
            seg_idx = np.zeros(n * 128, dtype=np.int16)
            seg_val = np.zeros(n * 128, dtype=np.float32)
            seg_dst = np.zeros(n * 128, dtype=np.float32)
            pos = 0
            for bb in range(p["w0"], p["w1"]):
                gi = (cc * NBLK + bb) * NCH + kk
                s0, s1 = starts[gi], starts[gi + 1]
                cnt = s1 - s0
                blk_len = int(caps[bb, kk]) * 128
                seg_idx[pos:pos + cnt] = lidx_s[s0:s1]
                # pad lanes re-read the last real row (val = 0 anyway):
                # repeated reads hit the open DRAM row instead of a
                # random-distance row 0
                seg_idx[pos + cnt:pos + blk_len] = \
                    seg_idx[pos + cnt - 1] if cnt > 0 else 0
                seg_val[pos:pos + cnt] = val_s[s0:s1]
                seg_dst[pos:pos + cnt] = d_s[s0:s1]
                pos += blk_len
            assert pos == n * 128
            # gather idx wrap: stream pos j -> (partition j%16, col j//16),
            # replicated into the 8 groups of 16 partitions
            iw = seg_idx.reshape(n * 8, 16).T          # [16, n*8]
            idx_mat[:, off * 8:(off + n) * 8] = np.tile(iw, (8, 1))
            # val/dest wrap: pos j -> (partition j%128, slot j//128)
            val_mat[:, off:off + n] = seg_val.reshape(n, 128).T
            dst_mat[:, off:off + n] = seg_dst.reshape(n, 128).T
            off += n
        per_core.append(dict(idx=idx_mat, val=val_mat, dst=dst_mat))

    return caps, plan, per_core, TOTS, A


# ---------------------------------------------------------------- kernel ---
def build_bass(cfg, caps, plan, TOTS):
    import concourse.bacc as bacc
    import concourse.bass as bass
    import concourse.mybir as mybir
    import concourse.tile as tile
    from concourse import library_config
    from concourse._compat import get_trn_type

    f32 = mybir.dt.float32
    bf16 = mybir.dt.bfloat16
    i16 = mybir.dt.int16
    D, NCH = cfg.D, cfg.NCH

    nc = bacc.Bacc(get_trn_type() or "TRN2", target_bir_lowering=False,
                   debug=False, num_swdge_queues=4)
    x_hbm = nc.dram_tensor("x", [cfg.CHUNK * NCH, D], f32,
                           kind="ExternalInput")
    w_hbm = nc.dram_tensor("w", [D, D], bf16, kind="ExternalInput")
    iota_hbm = nc.dram_tensor("iota", [128, 128], bf16, kind="ExternalInput")
    idx_hbm = nc.dram_tensor("idx", [128, TOTS * 8], i16,
                             kind="ExternalInput")
    val_hbm = nc.dram_tensor("val", [128, TOTS], f32, kind="ExternalInput")
    dst_hbm = nc.dram_tensor("dst", [128, TOTS], bf16, kind="ExternalInput")
    out_hbm = nc.dram_tensor("out", [cfg.CORE_ROWS, D], f32,
                             kind="ExternalOutput")

    # block -> (first (w,k,slot), last (w,k,slot)) for start/stop flags
    first_slot = {}
    last_slot = {}
    for pi, p in enumerate(plan):
        for s, bb in enumerate(p["slot_block"]):
            bb = int(bb)
            if bb not in first_slot:
                first_slot[bb] = (pi, s)
            last_slot[bb] = (pi, s)

    with tile.TileContext(nc) as tc:
        with (
            tc.tile_pool(name="const", bufs=1) as constp,
            tc.tile_pool(name="idxp", bufs=5) as idxp,
            tc.tile_pool(name="valp", bufs=5) as valp,
            tc.tile_pool(name="dstp", bufs=5) as dstp,
            tc.tile_pool(name="gp", bufs=5) as gp,
            tc.tile_pool(name="gvp", bufs=4) as gvp,
            tc.tile_pool(name="sp", bufs=4) as sp,
            tc.tile_pool(name="aggsb", bufs=4) as aggsbp,
            tc.tile_pool(name="stg", bufs=2) as stgp,
            tc.tile_pool(name="aggps", bufs=cfg.WINDOW,
                         space=bass.MemorySpace.PSUM) as aggpsp,
            tc.tile_pool(name="out2ps", bufs=2,
                         space=bass.MemorySpace.PSUM) as out2psp,
        ):
            nc.gpsimd.load_library(library_config.mlp)
            iota_sb = constp.tile([128, 128], bf16, tag="iota")
            w_sb = constp.tile([D, D], bf16, tag="w")
            nc.sync.dma_start(iota_sb[:], iota_hbm[:])
            nc.sync.dma_start(w_sb[:], w_hbm[:])

            nslots_max = max(p["nslots"] for p in plan)
            gq = 0  # SWDGE queue round-robin counter

            # touch every g buffer once so first-use lanes are never
            # uninitialized SBUF (NaN * 0 = NaN would poison the PSUM)
            for _ in range(5):
                gz = gp.tile([128, nslots_max, D], f32, tag="g")
                nc.vector.memset(gz[:], 0.0)

            for wi, (w0, w1) in enumerate(cfg.windows):
                nb = w1 - w0
                # one PSUM bank per block: accumulation-group state is
                # bank-wide, so two blocks must not share a bank
                aggps = [aggpsp.tile([64, 128], f32, tag="aggps",
                                     name=f"aggps_w{wi}_{i}")
                         for i in range(nb)]

                for kk in range(NCH):
                    p = plan[wi * NCH + kk]
                    n = p["nslots"]
                    if n == 0:
                        continue
                    off = sum(q["nslots"] for q in plan[:wi * NCH + kk])

                    idx_t = idxp.tile([128, nslots_max * 8], i16, tag="idx")
                    nc.sync.dma_start(idx_t[:, :n * 8],
                                      idx_hbm[:, off * 8:(off + n) * 8])
                    val_t = valp.tile([128, nslots_max], f32, tag="val")
                    nc.sync.dma_start(val_t[:, :n], val_hbm[:, off:off + n])
                    dst_t = dstp.tile([128, nslots_max], bf16, tag="dst")
                    nc.sync.dma_start(dst_t[:, :n], dst_hbm[:, off:off + n])
                    g_t = gp.tile([128, nslots_max, D], f32, tag="g")
                    # one gather call per dest block (finer calls
                    # interleave better across the 4 SWDGE queues than
                    # GMAX-packed ones; measured 649us vs 685us).
                    # Round-robin the queue so all four Q7 pairs generate
                    # descriptors in parallel.
                    q0 = 0
                    for bb in range(p["w0"], p["w1"]):
                        cb = int(caps[bb, kk])
                        if cb == 0:
                            continue
                        q1 = q0 + cb
                        nq = cb * 128
                        nc.gpsimd.dma_gather(
                            g_t[:, q0:q1, :],
                            x_hbm[kk * cfg.CHUNK:(kk + 1) * cfg.CHUNK, :],
                            idx_t[:, q0 * 8:q1 * 8], nq, nq, D,
                            queue_num=gq % 4)
                        gq += 1
                        q0 = q1
                    assert q0 == n

                    for s0 in range(0, n, cfg.SUB):
                        s1 = min(s0 + cfg.SUB, n)
                        ns = s1 - s0
                        gv_t = gvp.tile([128, cfg.SUB, D], bf16, tag="gv")
                        nc.vector.tensor_tensor(
                            gv_t[:, :ns, :], g_t[:, s0:s1, :],
                            val_t[:, s0:s1].unsqueeze(2)
                                .broadcast_to([128, ns, D]),
                            mybir.AluOpType.mult)
                        s_t = sp.tile([128, cfg.SUB, 128], bf16, tag="s")
                        nc.vector.tensor_tensor(
                            s_t[:, :ns, :],
                            dst_t[:, s0:s1].unsqueeze(2)
                                .broadcast_to([128, ns, 128]),
                            iota_sb[:, :].unsqueeze(1)
                                .broadcast_to([128, ns, 128]),
                            mybir.AluOpType.is_equal)
                        for s in range(s0, s1):
                            bb = int(p["slot_block"][s])
                            pi = wi * NCH + kk
                            st = first_slot[bb] == (pi, s)
                            sp_ = last_slot[bb] == (pi, s)
                            nc.tensor.matmul(
                                aggps[bb - w0][:, :],
                                gv_t[:, s - s0, :],
                                s_t[:, s - s0, :],
                                start=st, stop=sp_,
                                skip_group_check=True)

                # ---- flush window: apply W, stage, DMA out
                stg_t = stgp.tile([128, cfg.WINDOW, D], f32, tag="stg")
                out2 = out2psp.tile([128, cfg.WINDOW, D], f32, tag="out2")
                # flush copies run on the (otherwise idle) Scalar engine so
                # they don't stall the DVE gv/one-hot pipeline after each
                # window (DVE lag propagates into gather-buffer stalls)
                for bi in range(nb):
                    agg_sb = aggsbp.tile([64, 128], bf16, tag="aggsb",
                                         name=f"aggsb_w{wi}_{bi}")
                    nc.scalar.activation(agg_sb[:, :], aggps[bi][:, :],
                                         mybir.ActivationFunctionType.Copy)
                    nc.tensor.matmul(out2[:, bi, :],
                                     agg_sb[:, :], w_sb[:],
                                     start=True, stop=True,
                                     skip_group_check=True)
                nc.scalar.activation(stg_t[:, :nb, :], out2[:, :nb, :],
                                     mybir.ActivationFunctionType.Copy)
                # stg[p, b, f] -> out row (w0+b)*128+p, col f
                nc.sync.dma_start(
                    out_hbm[w0 * 128:w1 * 128, :]
                    .rearrange("(b p) f -> p b f", p=128),
                    stg_t[:, :nb, :])

    nc.compile()
    return nc


# ------------------------------------------------------------------- run ---
def _to_bf16(a):
    import ml_dtypes
    return a.astype(ml_dtypes.bfloat16)


def run(x, weight, edge_row, edge_col, edge_val, cfg=FULL, trace=False,
        trace_kwargs=None):
    from concourse.bass_utils import run_bass_kernel_spmd

    caps, plan, per_core, TOTS, A = preprocess(x, edge_row, edge_col,
                                               edge_val, cfg)
    nc = build_bass(cfg, caps, plan, TOTS)

    xpad = x
    if cfg.CHUNK * cfg.NCH > cfg.N:
        xpad = np.concatenate(
            [x, np.zeros((cfg.CHUNK * cfg.NCH - cfg.N, cfg.D),
                         dtype=np.float32)], axis=0)
    iota = _to_bf16(np.tile(np.arange(128, dtype=np.float32), (128, 1)))

    in_maps = []
    for cc in range(cfg.C):
        in_maps.append(dict(x=np.ascontiguousarray(xpad),
                            w=_to_bf16(weight),
                            iota=iota,
                            idx=per_core[cc]["idx"],
                            val=per_core[cc]["val"],
                            dst=_to_bf16(per_core[cc]["dst"])))
    kw = {}
    if trace:
        kw = dict(trace=True, trace_kwargs=trace_kwargs or {})
    res = run_bass_kernel_spmd(nc, in_maps, core_ids=list(range(cfg.C)), **kw)
    outs = [r["out"] for r in res.results]
    # un-permute: core c's position i holds global dest block A[i, c]
    full = np.zeros((cfg.C * cfg.NBLK * 128, cfg.D), dtype=np.float32)
    fullb = full.reshape(cfg.C * cfg.NBLK, 128, cfg.D)
    for cc in range(cfg.C):
        fullb[A[:, cc]] = outs[cc].reshape(cfg.NBLK, 128, cfg.D)
    return full[:cfg.N], res


def kernel(x, weight, edge_row, edge_col, edge_val):
    x = np.asarray(x, dtype=np.float32)
    weight = np.asarray(weight, dtype=np.float32)
    edge_row = np.asarray(edge_row, dtype=np.int32)
    edge_col = np.asarray(edge_col, dtype=np.int32)
    edge_val = np.asarray(edge_val, dtype=np.float32)
    out, _ = run(x, weight, edge_row, edge_col, edge_val, FULL)
    return out


# revision 41
# speedup vs baseline: 1.3169x; 1.0116x over previous
"""GCN layer (out = A @ x @ W, A sparse COO) on 8 Trainium2 NeuronCores.

Strategy (1D dest partitioning, x replicated):
  - Destinations (output rows) are sharded across the 8 cores; x is
    replicated to every core's HBM, the [64,64] weight is replicated.
  - Host-side preprocessing is pure indexing: edges are bucketed by
    (core, dest-block of 128 rows, source-chunk of 25000 rows), padded to a
    fixed per-segment slot capacity (max over cores, so one SPMD NEFF works
    for all 8 cores), and emitted as gather-index / value / dest-local
    streams.  Edges are sorted by source row within each segment for HBM
    locality.
  - Device per core: for each window of dest blocks, for each of the 4
    source chunks: dma_gather x rows (256B each) into SBUF -- gathers are
    issued round-robin over all 4 SWDGE queues so descriptor generation
    runs on all 8 GpSimd Q7 cores concurrently (4 pairs) instead of just
    pair 0; DVE multiplies the gathered rows by edge_val (bf16 out) and
    builds a one-hot [128 edges x 128 dest] bf16 matrix from dest-local
    ids (is_equal vs an iota row); PE matmuls (bf16, single-pass) accumulate
    aggT[64 feat, 128 dest] per block in fp32 PSUM across the window; at
    window end the [64,64] bf16 weight is applied per block
    (out_blk = (aggT)^T @ W) and fp32 results are DMA'd out.
  - Host concatenates the 8 output shards and truncates padding.
"""

import os
import numpy as np

# ---------------------------------------------------------------- config ---
class CFG:
    def __init__(self, n_nodes, d, n_cores, chunk, nchunks, nblk, window, sub):
        self.N = n_nodes
        self.D = d
        self.C = n_cores
        self.CHUNK = chunk          # x rows per gather chunk (< 32768 for int16)
        self.NCH = nchunks
        assert chunk * nchunks >= n_nodes
        self.NBLK = nblk            # dest blocks (of 128 rows) per core
        self.CORE_ROWS = 128 * nblk
        assert self.CORE_ROWS * n_cores >= n_nodes
        self.WINDOW = window        # blocks per window
        self.SUB = sub              # slots per DVE/PE subtile
        self.windows = [
            (w0, min(w0 + window, nblk)) for w0 in range(0, nblk, window)
        ]


FULL = CFG(n_nodes=100000, d=64, n_cores=8, chunk=25000, nchunks=4,
           nblk=98, window=7, sub=16)


# ---------------------------------------------------------- preprocessing ---
def preprocess(x, edge_row, edge_col, edge_val, cfg):
    """Bucket/pad edges; build per-core device input arrays.

    Returns (caps, plan, per_core_inputs):
      caps[b][k]   : slots (128-edge groups) for (block b, chunk k), shared
                     across cores.
      plan         : list over (w,k) of dicts with slot->block mapping etc.
      per_core     : list of dicts of numpy arrays keyed by dram tensor name.
    """
    C, NBLK, NCH = cfg.C, cfg.NBLK, cfg.NCH
    NPOS = C * NBLK
    r = edge_row.astype(np.int64)
    gblk = r // 128
    d = r % 128
    k = edge_col.astype(np.int64) // cfg.CHUNK
    lidx = (edge_col.astype(np.int64) % cfg.CHUNK).astype(np.int16)

    # Balance dest blocks across (core, position): slot capacities are
    # shared across cores (one SPMD NEFF, caps = max over the 8 blocks at
    # a position), so group blocks with similar per-chunk slot needs at
    # the same position (lexicographic sort of the cap vectors).
    gcounts = np.bincount(gblk * NCH + k, minlength=NPOS * NCH) \
        .reshape(NPOS, NCH)
    gcaps = -(-gcounts // 128)
    order_g = np.lexsort((gcaps[:, 3], gcaps[:, 2], gcaps[:, 1],
                          gcaps[:, 0]))
    A = order_g.reshape(NBLK, C)          # A[i, c] = global block id
    pos_c = np.empty(NPOS, np.int64)
    pos_i = np.empty(NPOS, np.int64)
    pos_c[A.reshape(-1)] = np.tile(np.arange(C), NBLK)
    pos_i[A.reshape(-1)] = np.repeat(np.arange(NBLK), C)
    caps = gcaps[A].max(axis=1)           # [NBLK, NCH]
    # every block must own >= 1 slot so its PSUM gets initialized
    empty = caps.sum(axis=1) == 0
    caps[empty, 0] = 1

    c = pos_c[gblk]
    b = pos_i[gblk]

    # order edges by (core, block, chunk, src) -- src-minor for DMA locality
    key = (((c * NBLK + b) * NCH + k) * cfg.CHUNK) + lidx
    order = np.argsort(key, kind="stable")
    seg_key = ((c * NBLK + b) * NCH + k)[order]
    counts = np.bincount(seg_key, minlength=C * NBLK * NCH).reshape(C, NBLK, NCH)
    assert (caps[None] >= np.ceil(counts / 128)).all()

    lidx_s = lidx[order]
    val_s = edge_val[order].astype(np.float32)
    d_s = d[order].astype(np.float32)

    # segment boundaries per (c,b,k) in the sorted stream
    starts = np.zeros(C * NBLK * NCH + 1, dtype=np.int64)
    np.cumsum(counts.reshape(-1), out=starts[1:])

    # fixed (w,k) segment sizes in slots
    plan = []
    for (w0, w1) in cfg.windows:
        for kk in range(NCH):
            nslots = int(caps[w0:w1, kk].sum())
            slot_block = np.repeat(np.arange(w0, w1), caps[w0:w1, kk])
            plan.append(dict(w0=w0, w1=w1, k=kk, nslots=nslots,
                             slot_block=slot_block))

    TOTS = sum(p["nslots"] for p in plan)

    per_core = []
    for cc in range(C):
        idx_mat = np.zeros((128, TOTS * 8), dtype=np.int16)
        val_mat = np.zeros((128, TOTS), dtype=np.float32)
        dst_mat = np.zeros((128, TOTS), dtype=np.float32)
        off = 0
        for p in plan:
            n = p["nslots"]
            if n == 0:
                continue
            kk = p["k"]
            # build padded segment arrays (length n*128)
            seg_idx = np.zeros(n * 128, dtype=np.int16)
            seg_val = np.zeros(n * 128, dtype=np.float32)
            seg_dst = np.zeros(n * 128, dtype=np.float32)
            pos = 0
            for bb in range(p["w0"], p["w1"]):
                gi = (cc * NBLK + bb) * NCH + kk
                s0, s1 = starts[gi], starts[gi + 1]
                cnt = s1 - s0
                blk_len = int(caps[bb, kk]) * 128
                seg_idx[pos:pos + cnt] = lidx_s[s0:s1]
                # pad lanes re-read the last real row (val = 0 anyway):
                # repeated reads hit the open DRAM row instead of a
                # random-distance row 0
                seg_idx[pos + cnt:pos + blk_len] = \
                    seg_idx[pos + cnt - 1] if cnt > 0 else 0
                seg_val[pos:pos + cnt] = val_s[s0:s1]
                seg_dst[pos:pos + cnt] = d_s[s0:s1]
                pos += blk_len
            assert pos == n * 128
            # gather idx wrap: stream pos j -> (partition j%16, col j//16),
            # replicated into the 8 groups of 16 partitions
            iw = seg_idx.reshape(n * 8, 16).T          # [16, n*8]
            idx_mat[:, off * 8:(off + n) * 8] = np.tile(iw, (8, 1))
            # val/dest wrap: pos j -> (partition j%128, slot j//128)
            val_mat[:, off:off + n] = seg_val.reshape(n, 128).T
            dst_mat[:, off:off + n] = seg_dst.reshape(n, 128).T
            off += n
        per_core.append(dict(idx=idx_mat, val=val_mat, dst=dst_mat))

    return caps, plan, per_core, TOTS, A


# ---------------------------------------------------------------- kernel ---
def build_bass(cfg, caps, plan, TOTS):
    import concourse.bacc as bacc
    import concourse.bass as bass
    import concourse.mybir as mybir
    import concourse.tile as tile
    from concourse import library_config
    from concourse._compat import get_trn_type

    f32 = mybir.dt.float32
    bf16 = mybir.dt.bfloat16
    i16 = mybir.dt.int16
    D, NCH = cfg.D, cfg.NCH

    nc = bacc.Bacc(get_trn_type() or "TRN2", target_bir_lowering=False,
                   debug=False, num_swdge_queues=4)
    x_hbm = nc.dram_tensor("x", [cfg.CHUNK * NCH, D], f32,
                           kind="ExternalInput")
    w_hbm = nc.dram_tensor("w", [D, D], bf16, kind="ExternalInput")
    iota_hbm = nc.dram_tensor("iota", [128, 128], bf16, kind="ExternalInput")
    idx_hbm = nc.dram_tensor("idx", [128, TOTS * 8], i16,
                             kind="ExternalInput")
    val_hbm = nc.dram_tensor("val", [128, TOTS], f32, kind="ExternalInput")
    dst_hbm = nc.dram_tensor("dst", [128, TOTS], bf16, kind="ExternalInput")
    out_hbm = nc.dram_tensor("out", [cfg.CORE_ROWS, D], f32,
                             kind="ExternalOutput")

    # block -> (first (w,k,slot), last (w,k,slot)) for start/stop flags
    first_slot = {}
    last_slot = {}
    for pi, p in enumerate(plan):
        for s, bb in enumerate(p["slot_block"]):
            bb = int(bb)
            if bb not in first_slot:
                first_slot[bb] = (pi, s)
            last_slot[bb] = (pi, s)

    with tile.TileContext(nc) as tc:
        with (
            tc.tile_pool(name="const", bufs=1) as constp,
            tc.tile_pool(name="idxp", bufs=5) as idxp,
            tc.tile_pool(name="valp", bufs=5) as valp,
            tc.tile_pool(name="dstp", bufs=5) as dstp,
            tc.tile_pool(name="gp", bufs=5) as gp,
            tc.tile_pool(name="gvp", bufs=4) as gvp,
            tc.tile_pool(name="sp", bufs=4) as sp,
            tc.tile_pool(name="aggsb", bufs=4) as aggsbp,
            tc.tile_pool(name="stg", bufs=2) as stgp,
            tc.tile_pool(name="aggps", bufs=cfg.WINDOW,
                         space=bass.MemorySpace.PSUM) as aggpsp,
            tc.tile_pool(name="out2ps", bufs=1,
                         space=bass.MemorySpace.PSUM) as out2psp,
        ):
            nc.gpsimd.load_library(library_config.mlp)
            iota_sb = constp.tile([128, 128], bf16, tag="iota")
            w_sb = constp.tile([D, D], bf16, tag="w")
            nc.sync.dma_start(iota_sb[:], iota_hbm[:])
            nc.sync.dma_start(w_sb[:], w_hbm[:])

            nslots_max = max(p["nslots"] for p in plan)
            gq = 0  # SWDGE queue round-robin counter

            # touch every g buffer once so first-use lanes are never
            # uninitialized SBUF (NaN * 0 = NaN would poison the PSUM)
            for _ in range(5):
                gz = gp.tile([128, nslots_max, D], f32, tag="g")
                nc.vector.memset(gz[:], 0.0)

            for wi, (w0, w1) in enumerate(cfg.windows):
                nb = w1 - w0
                # one PSUM bank per block: accumulation-group state is
                # bank-wide, so two blocks must not share a bank
                aggps = [aggpsp.tile([64, 128], f32, tag="aggps",
                                     name=f"aggps_w{wi}_{i}")
                         for i in range(nb)]

                for kk in range(NCH):
                    p = plan[wi * NCH + kk]
                    n = p["nslots"]
                    if n == 0:
                        continue
                    off = sum(q["nslots"] for q in plan[:wi * NCH + kk])

                    idx_t = idxp.tile([128, nslots_max * 8], i16, tag="idx")
                    nc.sync.dma_start(idx_t[:, :n * 8],
                                      idx_hbm[:, off * 8:(off + n) * 8])
                    val_t = valp.tile([128, nslots_max], f32, tag="val")
                    nc.sync.dma_start(val_t[:, :n], val_hbm[:, off:off + n])
                    dst_t = dstp.tile([128, nslots_max], bf16, tag="dst")
                    nc.sync.dma_start(dst_t[:, :n], dst_hbm[:, off:off + n])
                    g_t = gp.tile([128, nslots_max, D], f32, tag="g")
                    # one gather call per dest block (finer calls
                    # interleave better across the 4 SWDGE queues than
                    # GMAX-packed ones; measured 649us vs 685us).
                    # Round-robin the queue so all four Q7 pairs generate
                    # descriptors in parallel.
                    q0 = 0
                    for bb in range(p["w0"], p["w1"]):
                        cb = int(caps[bb, kk])
                        if cb == 0:
                            continue
                        q1 = q0 + cb
                        nq = cb * 128
                        nc.gpsimd.dma_gather(
                            g_t[:, q0:q1, :],
                            x_hbm[kk * cfg.CHUNK:(kk + 1) * cfg.CHUNK, :],
                            idx_t[:, q0 * 8:q1 * 8], nq, nq, D,
                            queue_num=gq % 4)
                        gq += 1
                        q0 = q1
                    assert q0 == n

                    for s0 in range(0, n, cfg.SUB):
                        s1 = min(s0 + cfg.SUB, n)
                        ns = s1 - s0
                        gv_t = gvp.tile([128, cfg.SUB, D], bf16, tag="gv")
                        nc.vector.tensor_tensor(
                            gv_t[:, :ns, :], g_t[:, s0:s1, :],
                            val_t[:, s0:s1].unsqueeze(2)
                                .broadcast_to([128, ns, D]),
                            mybir.AluOpType.mult)
                        s_t = sp.tile([128, cfg.SUB, 128], bf16, tag="s")
                        nc.vector.tensor_tensor(
                            s_t[:, :ns, :],
                            dst_t[:, s0:s1].unsqueeze(2)
                                .broadcast_to([128, ns, 128]),
                            iota_sb[:, :].unsqueeze(1)
                                .broadcast_to([128, ns, 128]),
                            mybir.AluOpType.is_equal)
                        for s in range(s0, s1):
                            bb = int(p["slot_block"][s])
                            pi = wi * NCH + kk
                            st = first_slot[bb] == (pi, s)
                            sp_ = last_slot[bb] == (pi, s)
                            nc.tensor.matmul(
                                aggps[bb - w0][:, :],
                                gv_t[:, s - s0, :],
                                s_t[:, s - s0, :],
                                start=st, stop=sp_,
                                skip_group_check=True)

                # ---- flush window: apply W, stage, DMA out
                stg_t = stgp.tile([128, cfg.WINDOW, D], f32, tag="stg")
                out2 = out2psp.tile([128, cfg.WINDOW, D], f32, tag="out2")
                # flush copies run on the (otherwise idle) Scalar engine so
                # they don't stall the DVE gv/one-hot pipeline after each
                # window (DVE lag propagates into gather-buffer stalls)
                for bi in range(nb):
                    agg_sb = aggsbp.tile([64, 128], bf16, tag="aggsb",
                                         name=f"aggsb_w{wi}_{bi}")
                    nc.scalar.activation(agg_sb[:, :], aggps[bi][:, :],
                                         mybir.ActivationFunctionType.Copy)
                    nc.tensor.matmul(out2[:, bi, :],
                                     agg_sb[:, :], w_sb[:],
                                     start=True, stop=True,
                                     skip_group_check=True)
                nc.scalar.activation(stg_t[:, :nb, :], out2[:, :nb, :],
                                     mybir.ActivationFunctionType.Copy)
                # stg[p, b, f] -> out row (w0+b)*128+p, col f
                nc.sync.dma_start(
                    out_hbm[w0 * 128:w1 * 128, :]
                    .rearrange("(b p) f -> p b f", p=128),
                    stg_t[:, :nb, :])

    nc.compile()
    return nc


# ------------------------------------------------------------------- run ---
def _to_bf16(a):
    import ml_dtypes
    return a.astype(ml_dtypes.bfloat16)


def run(x, weight, edge_row, edge_col, edge_val, cfg=FULL, trace=False,
        trace_kwargs=None):
    from concourse.bass_utils import run_bass_kernel_spmd

    caps, plan, per_core, TOTS, A = preprocess(x, edge_row, edge_col,
                                               edge_val, cfg)
    nc = build_bass(cfg, caps, plan, TOTS)

    xpad = x
    if cfg.CHUNK * cfg.NCH > cfg.N:
        xpad = np.concatenate(
            [x, np.zeros((cfg.CHUNK * cfg.NCH - cfg.N, cfg.D),
                         dtype=np.float32)], axis=0)
    iota = _to_bf16(np.tile(np.arange(128, dtype=np.float32), (128, 1)))

    in_maps = []
    for cc in range(cfg.C):
        in_maps.append(dict(x=np.ascontiguousarray(xpad),
                            w=_to_bf16(weight),
                            iota=iota,
                            idx=per_core[cc]["idx"],
                            val=per_core[cc]["val"],
                            dst=_to_bf16(per_core[cc]["dst"])))
    kw = {}
    if trace:
        kw = dict(trace=True, trace_kwargs=trace_kwargs or {})
    res = run_bass_kernel_spmd(nc, in_maps, core_ids=list(range(cfg.C)), **kw)
    outs = [r["out"] for r in res.results]
    # un-permute: core c's position i holds global dest block A[i, c]
    full = np.zeros((cfg.C * cfg.NBLK * 128, cfg.D), dtype=np.float32)
    fullb = full.reshape(cfg.C * cfg.NBLK, 128, cfg.D)
    for cc in range(cfg.C):
        fullb[A[:, cc]] = outs[cc].reshape(cfg.NBLK, 128, cfg.D)
    return full[:cfg.N], res


def kernel(x, weight, edge_row, edge_col, edge_val):
    x = np.asarray(x, dtype=np.float32)
    weight = np.asarray(weight, dtype=np.float32)
    edge_row = np.asarray(edge_row, dtype=np.int32)
    edge_col = np.asarray(edge_col, dtype=np.int32)
    edge_val = np.asarray(edge_val, dtype=np.float32)
    out, _ = run(x, weight, edge_row, edge_col, edge_val, FULL)
    return out
